# revision 2
# baseline (speedup 1.0000x reference)
"""BSplineKan layer kernel for 8 trn2 NeuronCores.

Math: out[b,o] = w_b*sum_i silu(x[b,i]) + w_s*sum_{i,k} bases_k(x[b,i]) * P[o,i,k]
with quadratic B-spline bases on uniform knots in [-1.125, 1.125], x ~ U[0,1).

Only bases k=5..12 are nonzero for x in [0,1). On uniform knots the spline
telescopes into truncated-power features:
    B2(t) = 0.5*[r(t) - 3r(t-1) + 3r(t-2) - r(t-3)],  r(t) = relu(t)^2
Folding per-(o,i) into host-precomputed weights, the device computes 10
feature planes per input element:
    v = x-1/2, v^2, relu^2(c_j - x) for the 3 interior knots left of 1/2,
    relu^2(x - c_j) for the 4 interior knots right of 1/2, silu(x)
(left-facing knots + centering balance magnitudes for fp32r's 11-bit
mantissa), then a single K=10*1024 fp32r matmul per output tile. The
constant term is folded into a host-side bias.

Sharding: 4-way batch x 2-way out_dim. Host pre-transposes x and pre-rounds
weights to fp32r; device output is (b, o) tiles DMA'd straight from PSUM.
"""

import numpy as np

import concourse.bass as bass
import concourse.bass_utils as _bu
import concourse.mybir as mybir
import concourse.tile as tile
from concourse import bacc
from concourse.bass_utils import run_bass_kernel_spmd

F32 = mybir.dt.float32
F32R = mybir.dt.float32r
AF = mybir.ActivationFunctionType
ALU = mybir.AluOpType

B, I, O = 2048, 1024, 1024
NB, NO = 4, 2              # batch-split x out-split
B_LOC, O_LOC = B // NB, O // NO   # 512, 512
H = 2.25 / 15.0            # knot spacing 0.15
# interior knots inside (0,1): c_j = j*H - 1.125 for j = 8..14
KNOTS = [j * H - 1.125 for j in range(8, 15)]
LEFT = KNOTS[:3]           # 0.075 0.225 0.375  -> relu^2(c - x)
RIGHT = KNOTS[3:]          # 0.525 0.675 0.825 0.975 -> relu^2(x - c)
N_PLANES = 10              # v, v^2, 3 left, 4 right, silu
K_TOT = N_PLANES * I

# enable walrus ldw-opt (pipelines fp32r weight loads behind streaming;
# default-off flag in bass_utils). Patch run_command rather than the
# hardcoded arg list.
_orig_run_command = _bu.run_command


def _run_command_ldwopt(argv, **kwargs):
    argv = ["--enable-ldw-opt=true" if a == "--enable-ldw-opt=false" else a
            for a in argv]
    return _orig_run_command(argv, **kwargs)


def _round_fp32r(a: np.ndarray) -> np.ndarray:
    """Round-to-nearest fp32 -> fp32r (11-bit mantissa, low 12 bits zero)."""
    u = a.astype(np.float32).view(np.uint32)
    u = (u + np.uint32(0x800)) & np.uint32(0xFFFFF000)
    return u.view(np.float32)


def fold_weights(P: np.ndarray, w_b: float, w_s: float):
    """Fold spline parameters into (K_TOT, O) device weights + (O,) bias."""
    Pd = P.astype(np.float64)
    O_, I_, _ = P.shape
    # G_j = coefficient of r_j = relu^2(u - j), u = (x + 1.125)/H, j = 5..14
    Pz = np.zeros((O_, I_, 18))
    Pz[:, :, 5:13] = Pd[:, :, 5:13]
    G = np.zeros((O_, I_, 15))
    for j in range(5, 15):
        G[:, :, j] = (0.5 * Pz[:, :, j] - 1.5 * Pz[:, :, j - 1]
                      + 1.5 * Pz[:, :, j - 2] - 0.5 * Pz[:, :, j - 3])
    c = np.array([j * H - 1.125 for j in range(15)])
    inv_h2 = 1.0 / (H * H)
    # ungated j=5,6,7 (u >= 7.5 always): (x - c_j)^2 / H^2 -> quadratic in x
    A = (G[:, :, 5] + G[:, :, 6] + G[:, :, 7]) * inv_h2
    Bq = -2.0 * (c[5] * G[:, :, 5] + c[6] * G[:, :, 6] + c[7] * G[:, :, 7]) * inv_h2
    Cq = (c[5] ** 2 * G[:, :, 5] + c[6] ** 2 * G[:, :, 6] + c[7] ** 2 * G[:, :, 7]) * inv_h2
    D = [G[:, :, 8 + t] * inv_h2 for t in range(7)]  # gated knots, x-units
    # flip left-of-center knots: D*relu^2(x-c) = D*(x-c)^2 - D*relu^2(c-x)
    left_w = []
    for t, cj in enumerate(KNOTS[:3]):
        A += D[t]
        Bq += -2.0 * cj * D[t]
        Cq += cj * cj * D[t]
        left_w.append(-D[t])
    right_w = [D[3 + t] for t in range(4)]
    # recenter quadratic at 1/2: A x^2 + B x + C = A v^2 + (B + A) v + const
    v_w = Bq + A
    v2_w = A
    bias = (Cq + 0.5 * Bq + 0.25 * A).sum(axis=1) * w_s          # (O,)
    silu_w = np.full((I_, O_), float(w_b))
    planes = [v_w, v2_w] + left_w + right_w                       # each (O, I)
    W = np.empty((N_PLANES * I_, O_), np.float32)
    for p, pw in enumerate(planes):
        W[p * I_:(p + 1) * I_, :] = (w_s * pw).T.astype(np.float32)
    W[9 * I_:10 * I_, :] = silu_w.astype(np.float32)
    return _round_fp32r(W), bias


def build_kernel(reps: int = 1):
    """Build the per-core Bass kernel (SPMD across 8 cores).

    reps > 1 wraps the whole body in a hardware loop for timing runs.
    """
    nc = bacc.Bacc("TRN2", target_bir_lowering=False, debug=False,
                   num_devices=8)
    xT_d = nc.dram_tensor("xT", [I, B_LOC], F32, kind="ExternalInput")
    W_d = nc.dram_tensor("Wf", [K_TOT, O_LOC], F32R, kind="ExternalInput")
    y_d = nc.dram_tensor("y", [B_LOC, O_LOC], F32, kind="ExternalOutput")

    n_ib = I // 128            # 8 i-tiles
    n_tb = B_LOC // 128        # 4 b-tiles
    PAIR = 2 * B_LOC           # feature FD covers two i-tiles

    with tile.TileContext(nc) as tc:
        with (
            tc.tile_pool(name="xt", bufs=1) as xt_pool,
            tc.tile_pool(name="wt", bufs=2) as wt_pool,
            tc.tile_pool(name="ft", bufs=2) as ft_pool,
            tc.tile_pool(name="sc", bufs=2) as sc_pool,
            tc.tile_pool(name="cn", bufs=1) as cn_pool,
            tc.tile_pool(name="ps", bufs=1, space="PSUM") as ps_pool,
        ):
            consts = cn_pool.tile([128, 8], F32, name="consts")
            nc.vector.memset(consts[:, 0:1], -0.5)      # v / v^2 bias
            psums = [ps_pool.tile([128, O_LOC], F32, tag=f"ps{t}", name=f"ps{t}")
                     for t in range(n_tb)]

            def body(_iv=None):
                xt = xt_pool.tile([128, n_ib * B_LOC], F32, name="xt")
                for t in range(n_ib):
                    nc.sync.dma_start(xt[:, t * B_LOC:(t + 1) * B_LOC],
                                      xT_d[t * 128:(t + 1) * 128, :])
                for pr in range(n_ib // 2):            # pairs of i-tiles
                    xs = xt[:, pr * PAIR:(pr + 1) * PAIR]
                    ft = ft_pool.tile([128, N_PLANES * PAIR], F32R,
                                      tag="ft", name=f"ft{pr}")

                    def plane(p):
                        return ft[:, p * PAIR:(p + 1) * PAIR]

                    # p0: v = x - 1/2 (DVE tensor_scalar, 2x mode)
                    nc.vector.tensor_scalar(plane(0), xs, 0.5, None, ALU.subtract)
                    # p1: v^2 (ACT square with bias)
                    nc.scalar.activation(plane(1), xs, AF.Square,
                                         bias=consts[:, 0:1], scale=1.0)
                    # p2-4: relu^2(c - x) via min(x - c, 0) then square
                    # p5-8: relu^2(x - c) via max(x - c, 0) then square
                    for t, cj in enumerate(LEFT + RIGHT):
                        gate = ALU.min if t < 3 else ALU.max
                        r = sc_pool.tile([128, PAIR], F32, tag="r", name=f"r{pr}_{t}")
                        nc.vector.tensor_scalar(r[:], xs, float(cj), 0.0,
                                                ALU.subtract, gate)
                        if t in (1, 2, 4):             # balance: some squares on DVE
                            nc.vector.tensor_tensor(plane(2 + t), r[:], r[:], ALU.mult)
                        else:
                            nc.scalar.activation(plane(2 + t), r[:], AF.Square)
                    # p9: silu
                    nc.scalar.activation(plane(9), xs, AF.Silu)

                    for s in range(2):                 # the two i-tiles of the pair
                        ib = pr * 2 + s
                        for j in range(N_PLANES):
                            w = wt_pool.tile([128, O_LOC], F32R, tag="w",
                                             name=f"w{ib}_{j}")
                            nc.sync.dma_start(
                                w[:], W_d[j * I + ib * 128:j * I + ib * 128 + 128, :])
                            for tb in range(n_tb):
                                nc.tensor.matmul(
                                    psums[tb][:],
                                    ft[:, j * PAIR + s * B_LOC + tb * 128:
                                       j * PAIR + s * B_LOC + (tb + 1) * 128],
                                    w[:],
                                    start=(ib == 0 and j == 0),
                                    stop=(ib == n_ib - 1 and j == N_PLANES - 1),
                                )
                for tb in range(n_tb):
                    o = sc_pool.tile([128, O_LOC], F32, tag="out", name=f"out{tb}")
                    nc.vector.tensor_copy(o[:], psums[tb][:])
                    nc.sync.dma_start(y_d[tb * 128:(tb + 1) * 128, :], o[:])

            if reps == 1:
                body()
            else:
                with tc.For_i(0, reps, 1) as iv:
                    body(iv)
    nc.compile()
    return nc


_cached_nc = None


def _get_nc():
    global _cached_nc
    if _cached_nc is None:
        _bu.run_command = _run_command_ldwopt
        _cached_nc = build_kernel(reps=1)
    return _cached_nc


def prepare_inputs(x, spline_parameters, w_b, w_s):
    """Host-side prep: returns (in_maps, bias) for the 8 cores."""
    x = np.ascontiguousarray(np.asarray(x, np.float32))
    P = np.asarray(spline_parameters, np.float32)
    W, bias = fold_weights(P, float(np.asarray(w_b)), float(np.asarray(w_s)))
    xT = np.ascontiguousarray(x.T)                     # (I, B)
    in_maps = []
    for c in range(8):
        bi, oi = c % NB, c // NB
        in_maps.append({
            "xT": np.ascontiguousarray(xT[:, bi * B_LOC:(bi + 1) * B_LOC]),
            "Wf": np.ascontiguousarray(W[:, oi * O_LOC:(oi + 1) * O_LOC]),
        })
    return in_maps, bias


def kernel(x, spline_parameters, w_b, w_s):
    in_maps, bias = prepare_inputs(x, spline_parameters, w_b, w_s)
    nc = _get_nc()
    res = run_bass_kernel_spmd(nc, in_maps, core_ids=list(range(8)))
    y = np.empty((B, O), np.float32)
    for c in range(8):
        bi, oi = c % NB, c // NB
        y[bi * B_LOC:(bi + 1) * B_LOC, oi * O_LOC:(oi + 1) * O_LOC] = (
            res.results[c]["y"]
            + bias[oi * O_LOC:(oi + 1) * O_LOC][None, :].astype(np.float32))
    return y


# revision 3
# speedup vs baseline: 1.4175x; 1.4175x over previous
"""BSplineKan layer kernel for 8 trn2 NeuronCores.

Math: out[b,o] = w_b*sum_i silu(x[b,i]) + w_s*sum_{i,k} bases_k(x[b,i]) * P[o,i,k]
with quadratic B-spline bases on 16 uniform knots over [-1.125, 1.125] and
x ~ U[0,1).

Only bases k=5..12 are nonzero for x in [0,1). On uniform knots the spline
telescopes into truncated-power features:
    B2(t) = 0.5*[r(t) - 3r(t-1) + 3r(t-2) - r(t-3)],  r(t) = relu(t)^2
Folding this per-(o,i) into host-precomputed weights, the device computes 10
feature planes per input element:
    v = x-1/2, v^2, relu^2(c_j - x) for the 3 interior knots left of 1/2,
    relu^2(x - c_j) for the 4 right of 1/2, and silu(x)
(left-facing knots + centering keep all plane magnitudes balanced, which is
what makes fp32r's 11-bit-mantissa rounding benign), followed by one fp32r
matmul with K = 10*I. The constant term becomes a host-side bias; silu rides
as a regular matmul plane with weight w_b so PSUM accumulates the silu-sum.

Sharding: contraction split — core c owns i in [128c, 128c+128). Each core
computes partial (2048, 1024) outputs; the host sums the 8 partials in fp64
and adds the bias. No device collectives.
"""

import numpy as np

import concourse.bass as bass
import concourse.bass_utils as _bu
import concourse.mybir as mybir
import concourse.tile as tile
from concourse import bacc
from concourse.bass_utils import run_bass_kernel_spmd

F32 = mybir.dt.float32
F32R = mybir.dt.float32r
AF = mybir.ActivationFunctionType
ALU = mybir.AluOpType

B, I, O = 2048, 1024, 1024
N_CORES = 8
I_LOC = I // N_CORES       # 128 contraction rows per core
H = 2.25 / 15.0            # knot spacing 0.15
# interior knots inside (0,1): c_j = j*H - 1.125 for j = 8..14
KNOTS = [j * H - 1.125 for j in range(8, 15)]
LEFT = KNOTS[:3]           # 0.075 0.225 0.375  -> relu^2(c - x)
RIGHT = KNOTS[3:]          # 0.525 0.675 0.825 0.975 -> relu^2(x - c)
N_PLANES = 10              # v, v^2, 3 left, 4 right, silu
N_TB = B // 128            # 16 batch tiles
N_OC = O // 512            # 2 output chunks of 512 (fp32 moving-dim max)

# enable walrus ldw-opt (pipelines fp32r weight loads behind streaming;
# default-off flag hardcoded in bass_utils). Patch run_command rather than
# the arg list.
_orig_run_command = _bu.run_command


def _run_command_ldwopt(argv, **kwargs):
    argv = ["--enable-ldw-opt=true" if a == "--enable-ldw-opt=false" else a
            for a in argv]
    return _orig_run_command(argv, **kwargs)


def _round_fp32r(a: np.ndarray) -> np.ndarray:
    """Round-to-nearest fp32 -> fp32r (11-bit mantissa, low 12 bits zero)."""
    u = np.ascontiguousarray(a, np.float32).view(np.uint32)
    u = (u + np.uint32(0x800)) & np.uint32(0xFFFFF000)
    return u.view(np.float32)


def fold_weights(P: np.ndarray, w_b: float, w_s: float):
    """Fold spline parameters into per-plane weights.

    Returns W (N_PLANES, I, O) float32 (fp32r-rounded) and bias (O,) float64.
    """
    Pd = P.astype(np.float64)
    O_, I_, _ = P.shape
    # G_j = coefficient of r_j = relu^2(u - j), u = (x + 1.125)/H, j = 5..14
    Pz = np.zeros((O_, I_, 18))
    Pz[:, :, 5:13] = Pd[:, :, 5:13]
    G = np.zeros((O_, I_, 15))
    for j in range(5, 15):
        G[:, :, j] = (0.5 * Pz[:, :, j] - 1.5 * Pz[:, :, j - 1]
                      + 1.5 * Pz[:, :, j - 2] - 0.5 * Pz[:, :, j - 3])
    c = np.array([j * H - 1.125 for j in range(15)])
    inv_h2 = 1.0 / (H * H)
    # ungated j=5,6,7 (u >= 7.5 always): (x - c_j)^2 / H^2 -> quadratic in x
    A = (G[:, :, 5] + G[:, :, 6] + G[:, :, 7]) * inv_h2
    Bq = -2.0 * (c[5] * G[:, :, 5] + c[6] * G[:, :, 6] + c[7] * G[:, :, 7]) * inv_h2
    Cq = (c[5] ** 2 * G[:, :, 5] + c[6] ** 2 * G[:, :, 6] + c[7] ** 2 * G[:, :, 7]) * inv_h2
    D = [G[:, :, 8 + t] * inv_h2 for t in range(7)]  # gated knots, x-units
    # flip left-of-center knots: D*relu^2(x-c) = D*(x-c)^2 - D*relu^2(c-x)
    left_w = []
    for t, cj in enumerate(LEFT):
        A += D[t]
        Bq += -2.0 * cj * D[t]
        Cq += cj * cj * D[t]
        left_w.append(-D[t])
    right_w = [D[3 + t] for t in range(4)]
    # recenter the quadratic at 1/2: A x^2 + B x + C = A v^2 + (A+B) v + const
    planes = [Bq + A, A] + left_w + right_w                   # each (O, I)
    bias = (Cq + 0.5 * Bq + 0.25 * A).sum(axis=1) * w_s       # (O,)
    W = np.empty((N_PLANES, I_, O_), np.float32)
    for p, pw in enumerate(planes):
        W[p] = (w_s * pw).T.astype(np.float32)
    W[9] = np.float32(w_b)                                    # silu plane
    return _round_fp32r(W), bias


def build_kernel(reps: int = 1):
    """Per-core Bass kernel (SPMD across 8 cores, contraction-split).

    reps > 1 wraps the body in a hardware loop for timing runs.
    """
    nc = bacc.Bacc("TRN2", target_bir_lowering=False, debug=False,
                   num_devices=N_CORES)
    xT_d = nc.dram_tensor("xT", [I_LOC, B], F32, kind="ExternalInput")
    W_d = nc.dram_tensor("Wf", [N_PLANES * I_LOC, O], F32R, kind="ExternalInput")
    y_d = nc.dram_tensor("y", [B, O], F32, kind="ExternalOutput")

    with tile.TileContext(nc) as tc:
        with (
            tc.tile_pool(name="xp", bufs=1) as x_pool,
            tc.tile_pool(name="wp", bufs=1) as w_pool,
            tc.tile_pool(name="fp", bufs=1) as f_pool,
            tc.tile_pool(name="sp", bufs=2) as s_pool,
            tc.tile_pool(name="op", bufs=3) as o_pool,
            tc.tile_pool(name="cp", bufs=1) as c_pool,
            tc.tile_pool(name="ps", bufs=2, space="PSUM") as ps_pool,
        ):
            consts = c_pool.tile([128, 1], F32, name="consts")
            nc.vector.memset(consts[:, 0:1], -0.5)

            def body(_iv=None):
                xt = x_pool.tile([128, B], F32, name="xt")
                nc.sync.dma_start(xt[:], xT_d[:])
                wt = w_pool.tile([128, N_PLANES * O], F32R, name="wt")
                w3 = wt[:].rearrange("p (j o) -> p j o", j=N_PLANES)
                src = W_d[:].rearrange("(j p) o -> p j o", p=128)
                nc.sync.dma_start(w3[:, 0:5, :], src[:, 0:5, :])
                nc.scalar.dma_start(w3[:, 5:10, :], src[:, 5:10, :])

                ft = f_pool.tile([128, N_PLANES * B], F32R, name="ft")

                def plane(p):
                    return ft[:, p * B:(p + 1) * B]

                # p0: v = x - 1/2 (DVE tensor_scalar, 2x fp32 mode)
                nc.vector.tensor_scalar(plane(0), xt[:], 0.5, None, ALU.subtract)
                # p1: v^2 (ACT square with bias)
                nc.scalar.activation(plane(1), xt[:], AF.Square,
                                     bias=consts[:, 0:1], scale=1.0)
                # p2-4: relu^2(c - x) via min(x - c, 0) then square
                # p5-8: relu^2(x - c) via max(x - c, 0) then square
                for t, cj in enumerate(LEFT + RIGHT):
                    gate = ALU.min if t < 3 else ALU.max
                    r = s_pool.tile([128, B], F32, tag="r", name=f"r{t}")
                    nc.vector.tensor_scalar(r[:], xt[:], float(cj), 0.0,
                                            ALU.subtract, gate)
                    nc.scalar.activation(plane(2 + t), r[:], AF.Square)
                # p9: silu
                nc.scalar.activation(plane(9), xt[:], AF.Silu)

                for tb in range(N_TB):
                    ot = o_pool.tile([128, O], F32, tag="ot", name=f"ot{tb}")
                    for oc in range(N_OC):
                        ps = ps_pool.tile([128, 512], F32, tag=f"ps{(tb * N_OC + oc) % 2}",
                                          name=f"ps{tb}_{oc}")
                        for j in range(N_PLANES):
                            nc.tensor.matmul(
                                ps[:],
                                ft[:, j * B + tb * 128:j * B + (tb + 1) * 128],
                                wt[:, j * O + oc * 512:j * O + oc * 512 + 512],
                                start=(j == 0), stop=(j == N_PLANES - 1),
                            )
                        # alternate PSUM->SBUF copies between DVE and ACT
                        if (tb * N_OC + oc) % 2 == 0:
                            nc.vector.tensor_copy(ot[:, oc * 512:(oc + 1) * 512], ps[:])
                        else:
                            nc.scalar.copy(ot[:, oc * 512:(oc + 1) * 512], ps[:])
                    eng = nc.sync if tb % 2 == 0 else nc.scalar
                    eng.dma_start(y_d[tb * 128:(tb + 1) * 128, :], ot[:])

            if reps == 1:
                body()
            else:
                with tc.For_i(0, reps, 1) as iv:
                    body(iv)
    nc.compile()
    return nc


_cached_nc = None


def _get_nc():
    global _cached_nc
    if _cached_nc is None:
        _bu.run_command = _run_command_ldwopt
        _cached_nc = build_kernel(reps=1)
    return _cached_nc


def prepare_inputs(x, spline_parameters, w_b, w_s):
    """Host-side prep: returns (in_maps, bias) for the 8 cores."""
    x = np.ascontiguousarray(np.asarray(x, np.float32))
    P = np.asarray(spline_parameters, np.float32)
    W, bias = fold_weights(P, float(np.asarray(w_b)), float(np.asarray(w_s)))
    xT = np.ascontiguousarray(x.T)                     # (I, B)
    in_maps = []
    for c in range(N_CORES):
        sl = slice(c * I_LOC, (c + 1) * I_LOC)
        in_maps.append({
            "xT": np.ascontiguousarray(xT[sl, :]),
            "Wf": np.ascontiguousarray(
                W[:, sl, :].reshape(N_PLANES * I_LOC, O)),
        })
    return in_maps, bias


def kernel(x, spline_parameters, w_b, w_s):
    in_maps, bias = prepare_inputs(x, spline_parameters, w_b, w_s)
    nc = _get_nc()
    res = run_bass_kernel_spmd(nc, in_maps, core_ids=list(range(N_CORES)))
    acc = np.zeros((B, O), np.float64)
    for c in range(N_CORES):
        acc += res.results[c]["y"]
    acc += bias[None, :]
    return acc.astype(np.float32)


# revision 5
# speedup vs baseline: 3.3529x; 2.3653x over previous
"""BSplineKan layer kernel for 8 trn2 NeuronCores.

Math: out[b,o] = w_b*sum_i silu(x[b,i]) + w_s*sum_{i,k} bases_k(x[b,i]) * P[o,i,k]
with quadratic B-spline bases on 16 uniform knots over [-1.125, 1.125] and
x ~ U[0,1).

Only bases k=5..12 are nonzero for x in [0,1). On uniform knots the spline
telescopes into truncated-power features:
    B2(t) = 0.5*[r(t) - 3r(t-1) + 3r(t-2) - r(t-3)],  r(t) = relu(t)^2
Folding this per-(o,i) into host-precomputed weights, the device computes 9
feature planes per input element:
    v = x-1/2, v^2, relu^2(c_j - x) for the 3 interior knots left of 1/2,
    relu^2(x - c_j) for the 4 right of 1/2
(left-facing knots + centering keep plane magnitudes balanced, which makes
fp32r's 11-bit-mantissa rounding benign), followed by one fp32r matmul with
K = 9*I. The constant term becomes a host-side bias. The silu sum is a
separate ACT pass over natural-layout x using accum_out, shipped to the host
as a tiny per-(core,b) column.

Sharding: contraction split — core c owns i in [128c, 128c+128). Each core
emits partial (2048, 1024) outputs in fp16; the host sums the 8 partials in
fp64 and adds bias + w_b * silu. No device collectives.
"""

import numpy as np

import concourse.bass as bass
import concourse.bass_utils as _bu
import concourse.mybir as mybir
import concourse.tile as tile
from concourse import bacc
from concourse.bass_utils import run_bass_kernel_spmd

F32 = mybir.dt.float32
F32R = mybir.dt.float32r
F16 = mybir.dt.float16
AF = mybir.ActivationFunctionType
ALU = mybir.AluOpType

B, I, O = 2048, 1024, 1024
N_CORES = 8
I_LOC = I // N_CORES       # 128 contraction rows per core
H = 2.25 / 15.0            # knot spacing 0.15
KNOTS = [j * H - 1.125 for j in range(8, 15)]   # interior knots in (0,1)
LEFT = KNOTS[:3]           # 0.075 0.225 0.375  -> relu^2(c - x)
RIGHT = KNOTS[3:]          # 0.525 0.675 0.825 0.975 -> relu^2(x - c)
N_PLANES = 9               # v, v^2, 3 left, 4 right
N_TB = B // 128            # 16 batch tiles
N_OC = O // 512            # 2 output chunks of 512 (fp32 moving-dim max)

# enable walrus ldw-opt (pipelines fp32r weight loads behind streaming;
# default-off flag hardcoded in bass_utils).
_orig_run_command = _bu.run_command


def _run_command_ldwopt(argv, **kwargs):
    argv = ["--enable-ldw-opt=true" if a == "--enable-ldw-opt=false" else a
            for a in argv]
    return _orig_run_command(argv, **kwargs)


def _round_fp32r(a: np.ndarray) -> np.ndarray:
    """Round-to-nearest fp32 -> fp32r (11-bit mantissa, low 12 bits zero)."""
    u = np.ascontiguousarray(a, np.float32).view(np.uint32)
    u = (u + np.uint32(0x800)) & np.uint32(0xFFFFF000)
    return u.view(np.float32)


def fold_weights(P: np.ndarray, w_s: float):
    """Fold spline parameters into per-plane weights.

    Returns W (N_PLANES, I, O) float32 (fp32r-rounded) and bias (O,) float64.
    """
    Pd = P.astype(np.float64)
    O_, I_, _ = P.shape
    # G_j = coefficient of r_j = relu^2(u - j), u = (x + 1.125)/H, j = 5..14
    Pz = np.zeros((O_, I_, 18))
    Pz[:, :, 5:13] = Pd[:, :, 5:13]
    G = np.zeros((O_, I_, 15))
    for j in range(5, 15):
        G[:, :, j] = (0.5 * Pz[:, :, j] - 1.5 * Pz[:, :, j - 1]
                      + 1.5 * Pz[:, :, j - 2] - 0.5 * Pz[:, :, j - 3])
    c = np.array([j * H - 1.125 for j in range(15)])
    inv_h2 = 1.0 / (H * H)
    # ungated j=5,6,7 (u >= 7.5 always): (x - c_j)^2 / H^2 -> quadratic in x
    A = (G[:, :, 5] + G[:, :, 6] + G[:, :, 7]) * inv_h2
    Bq = -2.0 * (c[5] * G[:, :, 5] + c[6] * G[:, :, 6] + c[7] * G[:, :, 7]) * inv_h2
    Cq = (c[5] ** 2 * G[:, :, 5] + c[6] ** 2 * G[:, :, 6] + c[7] ** 2 * G[:, :, 7]) * inv_h2
    D = [G[:, :, 8 + t] * inv_h2 for t in range(7)]  # gated knots, x-units
    # flip left-of-center knots: D*relu^2(x-c) = D*(x-c)^2 - D*relu^2(c-x)
    left_w = []
    for t, cj in enumerate(LEFT):
        A += D[t]
        Bq += -2.0 * cj * D[t]
        Cq += cj * cj * D[t]
        left_w.append(-D[t])
    right_w = [D[3 + t] for t in range(4)]
    # recenter the quadratic at 1/2: A x^2 + B x + C = A v^2 + (A+B) v + const
    planes = [Bq + A, A] + left_w + right_w                   # each (O, I)
    bias = (Cq + 0.5 * Bq + 0.25 * A).sum(axis=1) * w_s       # (O,)
    W = np.empty((N_PLANES, I_, O_), np.float32)
    for p, pw in enumerate(planes):
        W[p] = (w_s * pw).T.astype(np.float32)
    return _round_fp32r(W), bias


def build_kernel(reps: int = 1):
    """Per-core Bass kernel (SPMD across 8 cores, contraction-split).

    reps > 1 wraps the body in a hardware loop for timing runs.
    """
    nc = bacc.Bacc("TRN2", target_bir_lowering=False, debug=False,
                   num_devices=N_CORES)
    xT_d = nc.dram_tensor("xT", [I_LOC, B], F32, kind="ExternalInput")
    xN_d = nc.dram_tensor("xN", [B, I_LOC], F32, kind="ExternalInput")
    W_d = nc.dram_tensor("Wf", [N_PLANES * I_LOC, O], F32R, kind="ExternalInput")
    y_d = nc.dram_tensor("y", [B, O], F16, kind="ExternalOutput")
    s_d = nc.dram_tensor("ysilu", [128, N_TB], F32, kind="ExternalOutput")

    with tile.TileContext(nc) as tc:
        with (
            tc.tile_pool(name="xp", bufs=1) as x_pool,
            tc.tile_pool(name="wp", bufs=1) as w_pool,
            tc.tile_pool(name="fp", bufs=1) as f_pool,
            tc.tile_pool(name="sp", bufs=2) as s_pool,
            tc.tile_pool(name="op", bufs=3) as o_pool,
            tc.tile_pool(name="cp", bufs=1) as c_pool,
            tc.tile_pool(name="ps", bufs=2, space="PSUM") as ps_pool,
        ):
            consts = c_pool.tile([128, 1], F32, name="consts")
            nc.vector.memset(consts[:, 0:1], -0.5)

            def body(_iv=None):
                xt = x_pool.tile([128, B], F32, name="xt")
                nc.sync.dma_start(xt[:], xT_d[:])
                xn = x_pool.tile([128, N_TB * I_LOC], F32, name="xn")
                nc.scalar.dma_start(
                    xn[:].rearrange("p (t i) -> p t i", t=N_TB),
                    xN_d[:].rearrange("(t p) i -> p t i", p=128))
                wt = w_pool.tile([128, N_PLANES * O], F32R, name="wt")
                w3 = wt[:].rearrange("p (j o) -> p j o", j=N_PLANES)
                src = W_d[:].rearrange("(j p) o -> p j o", p=128)
                nc.sync.dma_start(w3[:, 0:4, :], src[:, 0:4, :])
                nc.scalar.dma_start(w3[:, 4:N_PLANES, :], src[:, 4:N_PLANES, :])

                ft = f_pool.tile([128, N_PLANES * B], F32R, name="ft")

                def plane(p):
                    return ft[:, p * B:(p + 1) * B]

                # p0: v = x - 1/2 (DVE tensor_scalar, 2x fp32 mode)
                nc.vector.tensor_scalar(plane(0), xt[:], 0.5, None, ALU.subtract)
                # p1: v^2 (ACT square with bias)
                nc.scalar.activation(plane(1), xt[:], AF.Square,
                                     bias=consts[:, 0:1], scale=1.0)
                # p2-4: relu^2(c - x) via min(x - c, 0) then square
                # p5-8: relu^2(x - c) via max(x - c, 0) then square
                for t, cj in enumerate(LEFT + RIGHT):
                    gate = ALU.min if t < 3 else ALU.max
                    r = s_pool.tile([128, B], F32, tag="r", name=f"r{t}")
                    nc.vector.tensor_scalar(r[:], xt[:], float(cj), 0.0,
                                            ALU.subtract, gate)
                    nc.scalar.activation(plane(2 + t), r[:], AF.Square)

                # silu sum over this core's i-slice, per batch row: ACT pass
                # on natural-layout x with accum_out
                acc = c_pool.tile([128, N_TB], F32, name="acc")
                for tb in range(N_TB):
                    sil = s_pool.tile([128, I_LOC], F32, tag="sil", name=f"sil{tb}")
                    nc.scalar.activation(
                        sil[:], xn[:, tb * I_LOC:(tb + 1) * I_LOC], AF.Silu,
                        accum_out=acc[:, tb:tb + 1])
                nc.sync.dma_start(s_d[:], acc[:])

                for tb in range(N_TB):
                    ot = o_pool.tile([128, O], F16, tag="ot", name=f"ot{tb}")
                    for oc in range(N_OC):
                        g = tb * N_OC + oc
                        ps = ps_pool.tile([128, 512], F32, tag=f"ps{g % 4}",
                                          name=f"ps{tb}_{oc}")
                        for j in range(N_PLANES):
                            nc.tensor.matmul(
                                ps[:],
                                ft[:, j * B + tb * 128:j * B + (tb + 1) * 128],
                                wt[:, j * O + oc * 512:j * O + oc * 512 + 512],
                                start=(j == 0), stop=(j == N_PLANES - 1),
                            )
                        # alternate PSUM->SBUF(bf16) copies between DVE and ACT
                        if g % 2 == 0:
                            nc.vector.tensor_copy(ot[:, oc * 512:(oc + 1) * 512], ps[:])
                        else:
                            nc.scalar.copy(ot[:, oc * 512:(oc + 1) * 512], ps[:])
                    eng = nc.sync if tb % 2 == 0 else nc.scalar
                    eng.dma_start(y_d[tb * 128:(tb + 1) * 128, :], ot[:])

            if reps == 1:
                body()
            else:
                with tc.For_i(0, reps, 1) as iv:
                    body(iv)
    nc.compile()
    return nc


_cached_nc = None


def _get_nc():
    global _cached_nc
    if _cached_nc is None:
        _bu.run_command = _run_command_ldwopt
        _cached_nc = build_kernel(reps=1)
    return _cached_nc


def prepare_inputs(x, spline_parameters, w_b, w_s):
    """Host-side prep: returns (in_maps, bias, w_b) for the 8 cores."""
    x = np.ascontiguousarray(np.asarray(x, np.float32))
    P = np.asarray(spline_parameters, np.float32)
    w_b = float(np.asarray(w_b))
    W, bias = fold_weights(P, float(np.asarray(w_s)))
    xT = np.ascontiguousarray(x.T)                     # (I, B)
    in_maps = []
    for c in range(N_CORES):
        sl = slice(c * I_LOC, (c + 1) * I_LOC)
        in_maps.append({
            "xT": np.ascontiguousarray(xT[sl, :]),
            "xN": np.ascontiguousarray(x[:, sl]),
            "Wf": np.ascontiguousarray(
                W[:, sl, :].reshape(N_PLANES * I_LOC, O)),
        })
    return in_maps, bias, w_b


def kernel(x, spline_parameters, w_b, w_s):
    in_maps, bias, w_b = prepare_inputs(x, spline_parameters, w_b, w_s)
    nc = _get_nc()
    res = run_bass_kernel_spmd(nc, in_maps, core_ids=list(range(N_CORES)))
    acc = np.zeros((B, O), np.float64)
    silu_sum = np.zeros((B,), np.float64)
    for c in range(N_CORES):
        acc += res.results[c]["y"].astype(np.float64)
        # ysilu[p, t] holds sum_i silu(x[t*128+p, i_slice])
        silu_sum += res.results[c]["ysilu"].T.reshape(B)
    acc += bias[None, :]
    acc += (w_b * silu_sum)[:, None]
    return acc.astype(np.float32)
